# revision 36
# baseline (speedup 1.0000x reference)
"""Trainium2 Bass kernel for gather + segment-sum (GNN sum-aggregator).

    out[s, :] = sum_{e : seg_ids[e] == s} features[neigh_idx[e], :]

Strategy (8 NeuronCores, SPMD single NEFF):
  - Shard the segment (destination-node) axis: core c owns segments
    [12500c, 12500(c+1)) and the contiguous slice of the sorted edge list
    that targets them. The feature table is replicated.
  - Features are split hi/lo into two bf16 halves packed side by side
    ([N, 128] bf16) so the mandatory 256B gather row is an exactly
    representable fp32 row as two bf16 matmul operands.
  - Windows of 128 segments; GW consecutive windows form a gather group.
    Per (group, bucket-of-32768-table-rows) ONE dma_gather fetches all the
    group's edges for that bucket (int16 SWDGE indices), amortizing the
    ~1us fixed SWDGE descriptor-gen cost that dominated the per-window
    design. Window runs inside the group stream are padded to 16 slots
    (max over the 8 cores, so the program is SPMD-static); pad slots
    gather spread-out real rows and carry rel=MISS so they contribute 0.
  - Segment-sum = matmul with per-block one-hot (lhsT = onehot[128 edges,
    128 segs] bf16, rhs = gathered [128, 64] bf16 hi half (HALF mode,
    rel err ~2e-3 vs the 2e-2 gate)) accumulated in PSUM per window.
    Blocks straddling a window boundary issue one matmul per overlapped
    window. One-hots are built on DVE, B=16 blocks per instruction, in
    [128, W, m] layout: iota varies along the middle dim and the rel
    operand broadcasts with a stride-0 MIDDLE dim, so every operand's
    last AP dim is packed — required for DVE's 2x 16-bit mode (a
    stride-0 LAST dim forces the 1 elem/cycle path, 2x slower).
  - Engine assignment: Pool=SWDGE gathers, DVE=one-hots, PE=matmuls,
    Act=PSUM->SBUF copies, SP=input loads + output stores. Engine queues
    are in-order, so no engine is ever given work that waits on another
    engine's completion mid-pipeline.
  - Measured wall: the SWDGE gather drain (~9.5 ns/row/queue x 4 queues
    ~= 520 us for 217K padded rows). Everything else overlaps under it.
"""

import math

import numpy as np
import ml_dtypes

N_NODES = 100000
N_EDGES = 1600000
D = 64
N_CORES = 8
SEGS_PER_CORE = N_NODES // N_CORES  # 12500
W = 128  # segments per window
NWIN = math.ceil(SEGS_PER_CORE / W)  # 98
SEG_PAD = NWIN * W  # 12544
BUCKET = 32768
NBUK = 4
MISS = 30000.0
GW = 7  # windows per gather group
NGRP = math.ceil(NWIN / GW)  # 14
HALF = True  # use only the hi-bf16 half of each gathered row (rel err ~2e-3)
OHSRC = "dve"  # "dve": build one-hots on DVE; "dma": host-precomputed fp8 stream
B = 16  # one-hot blocks per DVE build instruction (dve mode)
SPKT = False  # dma_gather single_packet flag
SHUF = True  # shuffle edges within runs (spread HBM accesses)


def _wrap_idxs(idx_flat: np.ndarray) -> np.ndarray:
    """[NI] -> [128, NI//16] int16 (16-partition wrap, replicated 8x)."""
    ni = idx_flat.shape[0]
    w = idx_flat.reshape(ni // 16, 16).T.astype(np.int16)
    return np.tile(w, (8, 1))


class _Plan:
    """Static (core-independent) program layout + per-core input arrays."""

    def __init__(self, neigh: np.ndarray, seg: np.ndarray):
        ebounds = np.searchsorted(seg, np.arange(N_CORES + 1) * SEGS_PER_CORE)
        counts = np.zeros((N_CORES, NWIN, NBUK), np.int64)
        per_core = []
        for c in range(N_CORES):
            e0, e1 = int(ebounds[c]), int(ebounds[c + 1])
            nidx = neigh[e0:e1]
            lseg = seg[e0:e1] - c * SEGS_PER_CORE
            win = lseg // W
            buk = nidx // BUCKET
            order = np.lexsort((nidx, buk, win))
            si = nidx[order]
            sr = (lseg - win * W).astype(np.float32)[order]
            key = (win * NBUK + buk)[order]
            cnt = np.bincount(key, minlength=NWIN * NBUK).reshape(NWIN, NBUK)
            counts[c] = cnt
            starts = np.concatenate([[0], np.cumsum(cnt.ravel())]).astype(np.int64)
            per_core.append((si, sr, starts))
        self.per_core = per_core
        # exact per-(window,bucket) cap = max count over the 8 cores.  Runs
        # inside a gather stream need no alignment — only the stream total
        # must be a multiple of 16 (index wrap) and 128 (block structure),
        # handled by the per-(group,bucket) tail pad.
        cap16 = counts.max(axis=0).astype(np.int64)  # [NWIN, NBUK]
        self.cap16 = cap16

        self.groups = [list(range(g * GW, min((g + 1) * GW, NWIN))) for g in range(NGRP)]
        self.P = np.zeros((NGRP, NBUK), np.int64)
        self.runoff = {}   # (g, k) -> {w: offset in stream}
        self.owner = {}    # (g, k) -> int32[P] window id or -1
        self.sched = []    # per group: list of (w, k, bl, col, start, stop)
        self.idxcol = np.zeros((NGRP, NBUK), np.int64)  # idx16 col offset (units of 1 col = 16 idxs)
        col = 0
        icol = 0
        for g in range(NGRP):
            wins = self.groups[g]
            for k in range(NBUK):
                caps = cap16[wins, k]
                offs = np.concatenate([[0], np.cumsum(caps)])
                S = int(offs[-1])
                Pgk = ((S + 127) // 128) * 128
                self.P[g, k] = Pgk
                self.runoff[(g, k)] = {w: int(offs[i]) for i, w in enumerate(wins)}
                own = np.full(Pgk, -1, np.int32)
                for i, w in enumerate(wins):
                    own[offs[i]:offs[i] + caps[i]] = w
                self.owner[(g, k)] = own
                self.idxcol[g, k] = icol
                icol += Pgk // 16
            glist = []
            for w in wins:
                wmms = []
                for k in range(NBUK):
                    ck = int(cap16[w, k])
                    if ck == 0:
                        continue
                    o0 = self.runoff[(g, k)][w]
                    for bl in range(o0 // 128, (o0 + ck - 1) // 128 + 1):
                        wmms.append((k, bl))
                if not wmms:
                    continue
                for i, (k, bl) in enumerate(wmms):
                    glist.append((w, k, bl, col, i == 0, i == len(wmms) - 1))
                    col += 1
            self.sched.append(glist)
        self.TOTC = icol
        self.NMM = col
        self.maxblk = [int(self.P[:, k].max()) // 128 for k in range(NBUK)]
        self.gcols = []  # per group: (col0, ncols)
        for g in range(NGRP):
            gl = self.sched[g]
            self.gcols.append((gl[0][3], len(gl)) if gl else (0, 0))
        self.maxmm = max(n for _, n in self.gcols)
        # the gather wall is the MAX queue's total rows: split big gathers
        # into two block-aligned halves and greedy-pack all pieces onto the
        # 4 SWDGE queues by accumulated row count
        self.pieces = {}  # (g, k) -> list of (bl0, bl1, queue)
        running = [0, 0, 0, 0]
        for g in range(NGRP):
            sizes = []
            for k in range(NBUK):
                nbl = int(self.P[g, k]) // 128
                if nbl == 0:
                    continue
                if nbl >= 16:
                    h = nbl // 2
                    sizes.append((k, 0, h))
                    sizes.append((k, h, nbl))
                else:
                    sizes.append((k, 0, nbl))
            for k, b0, b1 in sorted(sizes, key=lambda t: -(t[2] - t[1])):
                q = min(range(4), key=lambda i: running[i])
                self.pieces.setdefault((g, k), []).append((b0, b1, q))
                running[q] += (b1 - b0) * 128
        self.qrows = list(running)
        self.key = (self.TOTC, self.NMM, tuple(self.P.ravel().tolist()))

    def core_arrays(self, c: int):
        """Build idx16 [128, TOTC] int16 and ohst [128, NMM, W] fp8 for core c."""
        si, sr, starts = self.per_core[c]
        idx16 = np.zeros((128, self.TOTC), np.int16)
        rel = np.full((128, self.NMM), MISS, np.float32)
        for g in range(NGRP):
            wins = self.groups[g]
            gmms = self.sched[g]
            for k in range(NBUK):
                Pgk = int(self.P[g, k])
                if Pgk == 0:
                    continue
                tabsz = min(BUCKET, N_NODES - k * BUCKET)
                rng = np.random.default_rng(g * 1000003 + k * 7919 + c)
                idxs = ((np.arange(Pgk, dtype=np.int64) * 769 + g * 4099 + k * 131)
                        % tabsz)  # spread pad reads
                rels = np.full(Pgk, MISS, np.float32)
                for w in wins:
                    n = int(starts[w * NBUK + k + 1] - starts[w * NBUK + k])
                    if n == 0:
                        continue
                    a = int(starts[w * NBUK + k])
                    o0 = self.runoff[(g, k)][w]
                    if SHUF:
                        perm = rng.permutation(n)
                        idxs[o0:o0 + n] = si[a:a + n][perm] - k * BUCKET
                        rels[o0:o0 + n] = sr[a:a + n][perm]
                    else:
                        idxs[o0:o0 + n] = si[a:a + n] - k * BUCKET
                        rels[o0:o0 + n] = sr[a:a + n]
                co = int(self.idxcol[g, k])
                idx16[:, co:co + Pgk // 16] = _wrap_idxs(idxs)
                relblk = rels.reshape(-1, 128)
                ownblk = self.owner[(g, k)].reshape(-1, 128)
                for (w, kk, bl, colx, _s, _e) in gmms:
                    if kk != k:
                        continue
                    rel[:, colx] = np.where(ownblk[bl] == w, relblk[bl], MISS)
        if OHSRC == "dma":
            ohst = (rel[:, :, None] == np.arange(W, dtype=np.float32)) \
                .astype(ml_dtypes.float8_e4m3)
            return idx16, {"ohst": ohst}
        return idx16, {"rel": rel.astype(ml_dtypes.bfloat16)}


def _build_nc(plan: "_Plan", repeat: int = 1,
              parts: frozenset = frozenset({"gather", "onehot", "mm", "flush"}),
              gbufs: int = 5, obufs: int | None = None, pbufs: int = 8):
    if obufs is None:
        obufs = 3 if OHSRC == "dve" else 2
    import concourse.bass as bass
    import concourse.tile as tile
    from concourse import bacc, mybir

    tab_sizes = [min(BUCKET, N_NODES - k * BUCKET) for k in range(NBUK)]
    nc = bacc.Bacc("TRN2", target_bir_lowering=False, num_swdge_queues=4)
    tabs = [
        nc.dram_tensor(f"tab{k}", [tab_sizes[k], 2 * D], mybir.dt.bfloat16,
                       kind="ExternalInput")
        for k in range(NBUK)
    ]
    idx_d = nc.dram_tensor("idx16", [128, plan.TOTC], mybir.dt.int16,
                           kind="ExternalInput")
    if OHSRC == "dma":
        ohst_d = nc.dram_tensor("ohst", [128, plan.NMM, W], mybir.dt.float8e4,
                                kind="ExternalInput")
    else:
        rel_d = nc.dram_tensor("rel", [128, plan.NMM], mybir.dt.bfloat16,
                               kind="ExternalInput")
        iotar_d = nc.dram_tensor("iotar", [128, W, B], mybir.dt.bfloat16,
                                 kind="ExternalInput")
    out_d = nc.dram_tensor("out", [SEG_PAD, D], mybir.dt.float32,
                           kind="ExternalOutput")

    with tile.TileContext(nc) as tc:
        with (
            tc.tile_pool(name="const", bufs=1) as cpool,
            tc.tile_pool(name="g", bufs=gbufs) as gpool,
            tc.tile_pool(name="oh", bufs=obufs) as ohpool,
            tc.tile_pool(name="psum", bufs=pbufs, space="PSUM") as ppool,
            tc.tile_pool(name="flush", bufs=4) as fpool,
        ):
            idx_t = cpool.tile([128, plan.TOTC], mybir.dt.int16)
            nc.sync.dma_start(idx_t[:], idx_d[:])
            if OHSRC == "dve":
                rel_t = cpool.tile([128, plan.NMM], mybir.dt.bfloat16)
                nc.sync.dma_start(rel_t[:], rel_d[:])
                iotar_t = cpool.tile([128, W, B], mybir.dt.bfloat16)
                nc.sync.dma_start(iotar_t[:], iotar_d[:])

            g_shared = None
            if "gather" not in parts:
                g_shared = [
                    cpool.tile([128, plan.maxblk[k], 2 * D], mybir.dt.bfloat16,
                               name=f"gsh{k}")
                    for k in range(NBUK)
                ]
                for k in range(NBUK):
                    nc.vector.memset(g_shared[k][:], 0.25)
            oh_shared = None
            if "onehot" not in parts:
                if OHSRC == "dma":
                    oh_shared = cpool.tile([128, plan.maxmm, W],
                                           mybir.dt.float8e4, name="ohsh")
                else:
                    oh_shared = cpool.tile([128, W, B], mybir.dt.bfloat16,
                                           name="ohsh")
                nc.vector.memset(oh_shared[:], 0.0)

            def ohload(g):
                col0, ncols = plan.gcols[g]
                if ncols == 0 or "onehot" not in parts:
                    return oh_shared
                oh = ohpool.tile([128, plan.maxmm, W], mybir.dt.float8e4,
                                 name=f"oh{g}", tag="oh")
                nc.sync.dma_start(oh[:, :ncols, :],
                                  ohst_d[:, col0:col0 + ncols, :])
                return oh

            def body():
                # dma mode: ohst loads are software-pipelined one group ahead
                # so the SP sequencer issues them before it parks on the
                # previous group's output-store waits
                oh_tiles = {}
                if OHSRC == "dma" and ({"onehot", "mm"} & parts):
                    oh_tiles[0] = ohload(0)
                for g in range(NGRP):
                    g_ts = {}
                    for k in range(NBUK):
                        Pgk = int(plan.P[g, k])
                        if Pgk == 0:
                            continue
                        if g_shared is not None:
                            g_ts[k] = g_shared[k]
                            continue
                        gt = gpool.tile([128, plan.maxblk[k], 2 * D],
                                        mybir.dt.bfloat16, name=f"g{k}_{g}",
                                        tag=f"g{k}")
                        g_ts[k] = gt
                        co = int(plan.idxcol[g, k])
                        for (b0, b1, q) in plan.pieces[(g, k)]:
                            rows = (b1 - b0) * 128
                            nc.gpsimd.dma_gather(
                                gt[:, b0:b1, :],
                                tabs[k][:],
                                idx_t[:, co + b0 * 8:co + b1 * 8],
                                rows, rows, 2 * D,
                                single_packet=SPKT,
                                queue_num=q,
                            )
                    if not ({"onehot", "mm"} & parts):
                        continue
                    glist = plan.sched[g]
                    col0, ncols = plan.gcols[g]
                    if OHSRC == "dma":
                        if g + 1 < NGRP:
                            oh_tiles[g + 1] = ohload(g + 1)
                        if ncols == 0:
                            continue
                        oh_g = oh_tiles.pop(g)
                    if ncols == 0:
                        continue
                    psums = {}
                    rw = D if HALF else 2 * D
                    for c0 in range(0, ncols, B):
                        chunk = glist[c0:c0 + B]
                        m = len(chunk)
                        if OHSRC == "dve":
                            if "onehot" in parts:
                                oh = ohpool.tile([128, W, B], mybir.dt.bfloat16,
                                                 name=f"oh{g}_{c0}", tag="oh")
                                rel_b = rel_t[:, col0 + c0:col0 + c0 + m] \
                                    .unsqueeze(1).broadcast_to([128, W, m])
                                nc.vector.tensor_tensor(
                                    out=oh[:, :, :m], in0=iotar_t[:, :, :m],
                                    in1=rel_b, op=mybir.AluOpType.is_equal)
                            else:
                                oh = oh_shared
                        if "mm" not in parts:
                            continue
                        for j, (w, k, bl, colx, st, sp) in enumerate(chunk):
                            if st:
                                psums[w] = ppool.tile([W, rw], mybir.dt.float32,
                                                      name=f"ps{w}", tag="ps",
                                                      space="PSUM")
                            lhsT = (oh[:, :, j] if OHSRC == "dve"
                                    else oh_g[:, colx - col0, :])
                            nc.tensor.matmul(
                                psums[w][:], lhsT=lhsT,
                                rhs=g_ts[k][:, bl, :rw], start=st, stop=sp,
                            )
                            if sp and "flush" in parts:
                                cop = fpool.tile([W, rw], mybir.dt.float32,
                                                 name=f"cop{w}", tag="cop")
                                nc.scalar.copy(cop[:], psums[w][:])
                                if HALF:
                                    nc.sync.dma_start(
                                        out_d[w * W:(w + 1) * W, :], cop[:])
                                else:
                                    comb = fpool.tile([W, D], mybir.dt.float32,
                                                      name=f"comb{w}", tag="comb")
                                    nc.vector.tensor_tensor(
                                        out=comb[:], in0=cop[:, :D],
                                        in1=cop[:, D:], op=mybir.AluOpType.add,
                                    )
                                    nc.sync.dma_start(
                                        out_d[w * W:(w + 1) * W, :], comb[:])

            if repeat > 1:
                with tc.For_i(0, repeat, 1):
                    body()
            else:
                body()
    nc.finalize()
    return nc


class _SpmdRunner:
    """Compile once, execute the bass kernel across n_cores via PJRT shard_map."""

    def __init__(self, nc, n_cores: int):
        import jax
        import numpy as np
        from jax.experimental.shard_map import shard_map
        from jax.sharding import Mesh, NamedSharding, PartitionSpec
        import concourse.mybir as mybir
        from concourse.bass2jax import (
            _bass_exec_p, install_neuronx_cc_hook, partition_id_tensor,
        )

        install_neuronx_cc_hook()
        self.jax = jax
        self.n_cores = n_cores
        in_names, out_names, out_avals, zero_outs = [], [], [], []
        partition_name = nc.partition_id_tensor.name if nc.partition_id_tensor else None
        for alloc in nc.m.functions[0].allocations:
            if not isinstance(alloc, mybir.MemoryLocationSet):
                continue
            name = alloc.memorylocations[0].name
            if alloc.kind == "ExternalInput":
                if name != partition_name:
                    in_names.append(name)
            elif alloc.kind == "ExternalOutput":
                shape = tuple(alloc.tensor_shape)
                dtype = mybir.dt.np(alloc.dtype)
                out_names.append(name)
                out_avals.append(jax.core.ShapedArray(shape, dtype))
                zero_outs.append(np.zeros(shape, dtype))
        self.n_params = len(in_names)
        self.in_names = list(in_names)
        self.out_names = out_names
        self.out_avals = out_avals
        self.zero_outs = zero_outs
        all_in = in_names + out_names + ([partition_name] if partition_name else [])

        def _body(*args):
            operands = list(args)
            if partition_name is not None:
                operands.append(partition_id_tensor())
            outs = _bass_exec_p.bind(
                *operands,
                out_avals=tuple(out_avals),
                in_names=tuple(all_in),
                out_names=tuple(out_names),
                lowering_input_output_aliases=(),
                sim_require_finite=True,
                sim_require_nnan=True,
                nc=nc,
            )
            return tuple(outs)

        donate = tuple(range(self.n_params, self.n_params + len(out_names)))
        devices = jax.devices()[:n_cores]
        assert len(devices) >= n_cores, f"need {n_cores} cores, got {len(devices)}"
        self.mesh = Mesh(np.asarray(devices), ("core",))
        in_specs = (PartitionSpec("core"),) * (self.n_params + len(out_names))
        out_specs = (PartitionSpec("core"),) * len(out_names)
        self.fn = jax.jit(
            shard_map(_body, mesh=self.mesh, in_specs=in_specs, out_specs=out_specs,
                      check_rep=False),
            donate_argnums=donate,
            keep_unused=True,
        )
        self.sharding = NamedSharding(self.mesh, PartitionSpec("core"))

    def run(self, in_maps):
        np_ = np
        concat_in = [
            np_.concatenate([np_.asarray(in_maps[c][name]) for c in range(self.n_cores)],
                            axis=0)
            for name in self.in_names
        ]
        zeros = [np_.zeros((self.n_cores * z.shape[0], *z.shape[1:]), z.dtype)
                 for z in self.zero_outs]
        out = self.fn(*concat_in, *zeros)
        self.jax.block_until_ready(out)
        return [
            {n: np_.asarray(out[i]).reshape(self.n_cores, *self.out_avals[i].shape)[c]
             for i, n in enumerate(self.out_names)}
            for c in range(self.n_cores)
        ]


_CACHE = {}


def _get_runner(plan):
    if plan.key not in _CACHE:
        nc = _build_nc(plan)
        _CACHE[plan.key] = _SpmdRunner(nc, N_CORES)
    return _CACHE[plan.key]


def _prepare(features: np.ndarray, neigh: np.ndarray, seg: np.ndarray):
    """Returns (plan, in_maps) for the 8 cores."""
    hi = features.astype(ml_dtypes.bfloat16)
    lo = (features - hi.astype(np.float32)).astype(ml_dtypes.bfloat16)
    packed = np.ascontiguousarray(np.concatenate([hi, lo], axis=1))
    plan = _Plan(neigh, seg)
    tabs = {f"tab{k}": packed[k * BUCKET: min((k + 1) * BUCKET, N_NODES)]
            for k in range(NBUK)}
    iotar = np.tile(np.arange(W, dtype=np.float32)[None, :, None],
                    (128, 1, B)).astype(ml_dtypes.bfloat16)
    in_maps = []
    for c in range(N_CORES):
        idx16, extra = plan.core_arrays(c)
        m = dict(tabs)
        m["idx16"] = idx16
        m.update(extra)
        if OHSRC == "dve":
            m["iotar"] = iotar
        in_maps.append(m)
    return plan, in_maps


def kernel(features: np.ndarray, neigh_idx: np.ndarray, seg_ids: np.ndarray,
           ) -> np.ndarray:
    features = np.ascontiguousarray(np.asarray(features, dtype=np.float32))
    neigh = np.asarray(neigh_idx).astype(np.int64)
    seg = np.asarray(seg_ids).astype(np.int64)
    assert features.shape == (N_NODES, D)
    assert neigh.shape == (N_EDGES,) and seg.shape == (N_EDGES,)

    plan, in_maps = _prepare(features, neigh, seg)
    runner = _get_runner(plan)
    results = runner.run(in_maps)
    out = np.empty((N_NODES, D), np.float32)
    for c in range(N_CORES):
        out[c * SEGS_PER_CORE: (c + 1) * SEGS_PER_CORE] = \
            results[c]["out"][:SEGS_PER_CORE]
    return out


# revision 37
# speedup vs baseline: 1.0453x; 1.0453x over previous
"""Trainium2 Bass kernel for gather + segment-sum (GNN sum-aggregator).

    out[s, :] = sum_{e : seg_ids[e] == s} features[neigh_idx[e], :]

Strategy (8 NeuronCores, SPMD single NEFF):
  - Shard the segment (destination-node) axis: core c owns segments
    [12500c, 12500(c+1)) and the contiguous slice of the sorted edge list
    that targets them. The feature table is replicated.
  - Features are split hi/lo into two bf16 halves packed side by side
    ([N, 128] bf16) so the mandatory 256B gather row is an exactly
    representable fp32 row as two bf16 matmul operands.
  - Windows of 128 segments; GW consecutive windows form a gather group.
    Per (group, bucket-of-32768-table-rows) ONE dma_gather fetches all the
    group's edges for that bucket (int16 SWDGE indices), amortizing the
    ~1us fixed SWDGE descriptor-gen cost that dominated the per-window
    design. Window runs inside the group stream are padded to 16 slots
    (max over the 8 cores, so the program is SPMD-static); pad slots
    gather spread-out real rows and carry rel=MISS so they contribute 0.
  - Segment-sum = matmul with per-block one-hot (lhsT = onehot[128 edges,
    128 segs] bf16, rhs = gathered [128, 64] bf16 hi half (HALF mode,
    rel err ~2e-3 vs the 2e-2 gate)) accumulated in PSUM per window.
    Blocks straddling a window boundary issue one matmul per overlapped
    window. One-hots are built on DVE, B=16 blocks per instruction, in
    [128, W, m] layout: iota varies along the middle dim and the rel
    operand broadcasts with a stride-0 MIDDLE dim, so every operand's
    last AP dim is packed — required for DVE's 2x 16-bit mode (a
    stride-0 LAST dim forces the 1 elem/cycle path, 2x slower).
  - Engine assignment: Pool=SWDGE gathers, DVE=one-hots, PE=matmuls,
    Act=PSUM->SBUF copies, SP=input loads + output stores. Engine queues
    are in-order, so no engine is ever given work that waits on another
    engine's completion mid-pipeline.
  - Measured wall: the SWDGE gather drain (~9.5 ns/row/queue x 4 queues
    ~= 520 us for 217K padded rows). Everything else overlaps under it.
"""

import math

import numpy as np
import ml_dtypes

N_NODES = 100000
N_EDGES = 1600000
D = 64
N_CORES = 8
SEGS_PER_CORE = N_NODES // N_CORES  # 12500
W = 128  # segments per window
NWIN = math.ceil(SEGS_PER_CORE / W)  # 98
SEG_PAD = NWIN * W  # 12544
BUCKET = 32768
NBUK = 4
MISS = 30000.0
GW = 7  # windows per gather group
NGRP = math.ceil(NWIN / GW)  # 14
HALF = True  # use only the hi-bf16 half of each gathered row (rel err ~2e-3)
OHSRC = "dve"  # "dve": build one-hots on DVE; "dma": host-precomputed fp8 stream
B = 16  # one-hot blocks per DVE build instruction (dve mode)
SPKT = False  # dma_gather single_packet flag
SHUF = True  # shuffle edges within runs (spread HBM accesses)


def _wrap_idxs(idx_flat: np.ndarray) -> np.ndarray:
    """[NI] -> [128, NI//16] int16 (16-partition wrap, replicated 8x)."""
    ni = idx_flat.shape[0]
    w = idx_flat.reshape(ni // 16, 16).T.astype(np.int16)
    return np.tile(w, (8, 1))


class _Plan:
    """Static (core-independent) program layout + per-core input arrays."""

    def __init__(self, neigh: np.ndarray, seg: np.ndarray):
        ebounds = np.searchsorted(seg, np.arange(N_CORES + 1) * SEGS_PER_CORE)
        counts = np.zeros((N_CORES, NWIN, NBUK), np.int64)
        per_core = []
        for c in range(N_CORES):
            e0, e1 = int(ebounds[c]), int(ebounds[c + 1])
            nidx = neigh[e0:e1]
            lseg = seg[e0:e1] - c * SEGS_PER_CORE
            win = lseg // W
            buk = nidx // BUCKET
            order = np.lexsort((nidx, buk, win))
            si = nidx[order]
            sr = (lseg - win * W).astype(np.float32)[order]
            key = (win * NBUK + buk)[order]
            cnt = np.bincount(key, minlength=NWIN * NBUK).reshape(NWIN, NBUK)
            counts[c] = cnt
            starts = np.concatenate([[0], np.cumsum(cnt.ravel())]).astype(np.int64)
            per_core.append((si, sr, starts))
        self.per_core = per_core
        # exact per-(window,bucket) cap = max count over the 8 cores.  Runs
        # inside a gather stream need no alignment — only the stream total
        # must be a multiple of 16 (index wrap) and 128 (block structure),
        # handled by the per-(group,bucket) tail pad.
        cap16 = counts.max(axis=0).astype(np.int64)  # [NWIN, NBUK]
        self.cap16 = cap16

        self.groups = [list(range(g * GW, min((g + 1) * GW, NWIN))) for g in range(NGRP)]
        self.P = np.zeros((NGRP, NBUK), np.int64)
        self.runoff = {}   # (g, k) -> {w: offset in stream}
        self.owner = {}    # (g, k) -> int32[P] window id or -1
        self.sched = []    # per group: list of (w, k, bl, col, start, stop)
        self.idxcol = np.zeros((NGRP, NBUK), np.int64)  # idx16 col offset (units of 1 col = 16 idxs)
        col = 0
        icol = 0
        for g in range(NGRP):
            wins = self.groups[g]
            for k in range(NBUK):
                caps = cap16[wins, k]
                offs = np.concatenate([[0], np.cumsum(caps)])
                S = int(offs[-1])
                Pgk = ((S + 127) // 128) * 128
                self.P[g, k] = Pgk
                self.runoff[(g, k)] = {w: int(offs[i]) for i, w in enumerate(wins)}
                own = np.full(Pgk, -1, np.int32)
                for i, w in enumerate(wins):
                    own[offs[i]:offs[i] + caps[i]] = w
                self.owner[(g, k)] = own
                self.idxcol[g, k] = icol
                icol += Pgk // 16
            glist = []
            for w in wins:
                wmms = []
                for k in range(NBUK):
                    ck = int(cap16[w, k])
                    if ck == 0:
                        continue
                    o0 = self.runoff[(g, k)][w]
                    for bl in range(o0 // 128, (o0 + ck - 1) // 128 + 1):
                        wmms.append((k, bl))
                if not wmms:
                    continue
                for i, (k, bl) in enumerate(wmms):
                    glist.append((w, k, bl, col, i == 0, i == len(wmms) - 1))
                    col += 1
            self.sched.append(glist)
        self.TOTC = icol
        self.NMM = col
        self.maxblk = [int(self.P[:, k].max()) // 128 for k in range(NBUK)]
        self.gcols = []  # per group: (col0, ncols)
        for g in range(NGRP):
            gl = self.sched[g]
            self.gcols.append((gl[0][3], len(gl)) if gl else (0, 0))
        self.maxmm = max(n for _, n in self.gcols)
        # the gather wall is the MAX queue's total rows: split big gathers
        # into two block-aligned halves and greedy-pack all pieces onto the
        # 4 SWDGE queues by accumulated row count
        self.pieces = {}  # (g, k) -> list of (bl0, bl1, queue)
        running = [0, 0, 0, 0]
        for g in range(NGRP):
            sizes = []
            for k in range(NBUK):
                nbl = int(self.P[g, k]) // 128
                if nbl == 0:
                    continue
                if nbl >= 16:
                    h = nbl // 2
                    sizes.append((k, 0, h))
                    sizes.append((k, h, nbl))
                else:
                    sizes.append((k, 0, nbl))
            for k, b0, b1 in sorted(sizes, key=lambda t: -(t[2] - t[1])):
                q = min(range(4), key=lambda i: running[i])
                self.pieces.setdefault((g, k), []).append((b0, b1, q))
                running[q] += (b1 - b0) * 128
        self.qrows = list(running)
        self.key = (self.TOTC, self.NMM, tuple(self.P.ravel().tolist()))

    def core_arrays(self, c: int):
        """Build idx16 [128, TOTC] int16 and ohst [128, NMM, W] fp8 for core c."""
        si, sr, starts = self.per_core[c]
        idx16 = np.zeros((128, self.TOTC), np.int16)
        rel = np.full((128, self.NMM), MISS, np.float32)
        for g in range(NGRP):
            wins = self.groups[g]
            gmms = self.sched[g]
            for k in range(NBUK):
                Pgk = int(self.P[g, k])
                if Pgk == 0:
                    continue
                tabsz = min(BUCKET, N_NODES - k * BUCKET)
                rng = np.random.default_rng(g * 1000003 + k * 7919 + c)
                idxs = ((np.arange(Pgk, dtype=np.int64) * 769 + g * 4099 + k * 131)
                        % tabsz)  # spread pad reads
                rels = np.full(Pgk, MISS, np.float32)
                for w in wins:
                    n = int(starts[w * NBUK + k + 1] - starts[w * NBUK + k])
                    if n == 0:
                        continue
                    a = int(starts[w * NBUK + k])
                    o0 = self.runoff[(g, k)][w]
                    if SHUF:
                        perm = rng.permutation(n)
                        idxs[o0:o0 + n] = si[a:a + n][perm] - k * BUCKET
                        rels[o0:o0 + n] = sr[a:a + n][perm]
                    else:
                        idxs[o0:o0 + n] = si[a:a + n] - k * BUCKET
                        rels[o0:o0 + n] = sr[a:a + n]
                co = int(self.idxcol[g, k])
                idx16[:, co:co + Pgk // 16] = _wrap_idxs(idxs)
                relblk = rels.reshape(-1, 128)
                ownblk = self.owner[(g, k)].reshape(-1, 128)
                for (w, kk, bl, colx, _s, _e) in gmms:
                    if kk != k:
                        continue
                    rel[:, colx] = np.where(ownblk[bl] == w, relblk[bl], MISS)
        if OHSRC == "dma":
            ohst = (rel[:, :, None] == np.arange(W, dtype=np.float32)) \
                .astype(ml_dtypes.float8_e4m3)
            return idx16, {"ohst": ohst}
        return idx16, {"rel": rel.astype(ml_dtypes.bfloat16)}


def _build_nc(plan: "_Plan", repeat: int = 1,
              parts: frozenset = frozenset({"gather", "onehot", "mm", "flush"}),
              gbufs: int = 4, obufs: int | None = None, pbufs: int = 8):
    if obufs is None:
        obufs = 6 if OHSRC == "dve" else 2
    import concourse.bass as bass
    import concourse.tile as tile
    from concourse import bacc, mybir

    tab_sizes = [min(BUCKET, N_NODES - k * BUCKET) for k in range(NBUK)]
    nc = bacc.Bacc("TRN2", target_bir_lowering=False, num_swdge_queues=4)
    tabs = [
        nc.dram_tensor(f"tab{k}", [tab_sizes[k], 2 * D], mybir.dt.bfloat16,
                       kind="ExternalInput")
        for k in range(NBUK)
    ]
    idx_d = nc.dram_tensor("idx16", [128, plan.TOTC], mybir.dt.int16,
                           kind="ExternalInput")
    if OHSRC == "dma":
        ohst_d = nc.dram_tensor("ohst", [128, plan.NMM, W], mybir.dt.float8e4,
                                kind="ExternalInput")
    else:
        rel_d = nc.dram_tensor("rel", [128, plan.NMM], mybir.dt.bfloat16,
                               kind="ExternalInput")
        iotar_d = nc.dram_tensor("iotar", [128, W, B], mybir.dt.bfloat16,
                                 kind="ExternalInput")
    out_d = nc.dram_tensor("out", [SEG_PAD, D], mybir.dt.float32,
                           kind="ExternalOutput")

    with tile.TileContext(nc) as tc:
        with (
            tc.tile_pool(name="const", bufs=1) as cpool,
            tc.tile_pool(name="g", bufs=gbufs) as gpool,
            tc.tile_pool(name="oh", bufs=obufs) as ohpool,
            tc.tile_pool(name="psum", bufs=pbufs, space="PSUM") as ppool,
            tc.tile_pool(name="flush", bufs=4) as fpool,
        ):
            idx_t = cpool.tile([128, plan.TOTC], mybir.dt.int16)
            nc.sync.dma_start(idx_t[:], idx_d[:])
            if OHSRC == "dve":
                rel_t = cpool.tile([128, plan.NMM], mybir.dt.bfloat16)
                nc.sync.dma_start(rel_t[:], rel_d[:])
                iotar_t = cpool.tile([128, W, B], mybir.dt.bfloat16)
                nc.sync.dma_start(iotar_t[:], iotar_d[:])

            g_shared = None
            if "gather" not in parts:
                g_shared = [
                    cpool.tile([128, plan.maxblk[k], 2 * D], mybir.dt.bfloat16,
                               name=f"gsh{k}")
                    for k in range(NBUK)
                ]
                for k in range(NBUK):
                    nc.vector.memset(g_shared[k][:], 0.25)
            oh_shared = None
            if "onehot" not in parts:
                if OHSRC == "dma":
                    oh_shared = cpool.tile([128, plan.maxmm, W],
                                           mybir.dt.float8e4, name="ohsh")
                else:
                    oh_shared = cpool.tile([128, W, B], mybir.dt.bfloat16,
                                           name="ohsh")
                nc.vector.memset(oh_shared[:], 0.0)

            def ohload(g):
                col0, ncols = plan.gcols[g]
                if ncols == 0 or "onehot" not in parts:
                    return oh_shared
                oh = ohpool.tile([128, plan.maxmm, W], mybir.dt.float8e4,
                                 name=f"oh{g}", tag="oh")
                nc.sync.dma_start(oh[:, :ncols, :],
                                  ohst_d[:, col0:col0 + ncols, :])
                return oh

            def body():
                # dma mode: ohst loads are software-pipelined one group ahead
                # so the SP sequencer issues them before it parks on the
                # previous group's output-store waits
                oh_tiles = {}
                if OHSRC == "dma" and ({"onehot", "mm"} & parts):
                    oh_tiles[0] = ohload(0)
                for g in range(NGRP):
                    g_ts = {}
                    for k in range(NBUK):
                        Pgk = int(plan.P[g, k])
                        if Pgk == 0:
                            continue
                        if g_shared is not None:
                            g_ts[k] = g_shared[k]
                            continue
                        gt = gpool.tile([128, plan.maxblk[k], 2 * D],
                                        mybir.dt.bfloat16, name=f"g{k}_{g}",
                                        tag=f"g{k}")
                        g_ts[k] = gt
                        co = int(plan.idxcol[g, k])
                        for (b0, b1, q) in plan.pieces[(g, k)]:
                            rows = (b1 - b0) * 128
                            nc.gpsimd.dma_gather(
                                gt[:, b0:b1, :],
                                tabs[k][:],
                                idx_t[:, co + b0 * 8:co + b1 * 8],
                                rows, rows, 2 * D,
                                single_packet=SPKT,
                                queue_num=q,
                            )
                    if not ({"onehot", "mm"} & parts):
                        continue
                    glist = plan.sched[g]
                    col0, ncols = plan.gcols[g]
                    if OHSRC == "dma":
                        if g + 1 < NGRP:
                            oh_tiles[g + 1] = ohload(g + 1)
                        if ncols == 0:
                            continue
                        oh_g = oh_tiles.pop(g)
                    if ncols == 0:
                        continue
                    psums = {}
                    rw = D if HALF else 2 * D
                    for c0 in range(0, ncols, B):
                        chunk = glist[c0:c0 + B]
                        m = len(chunk)
                        if OHSRC == "dve":
                            if "onehot" in parts:
                                oh = ohpool.tile([128, W, B], mybir.dt.bfloat16,
                                                 name=f"oh{g}_{c0}", tag="oh")
                                rel_b = rel_t[:, col0 + c0:col0 + c0 + m] \
                                    .unsqueeze(1).broadcast_to([128, W, m])
                                nc.vector.tensor_tensor(
                                    out=oh[:, :, :m], in0=iotar_t[:, :, :m],
                                    in1=rel_b, op=mybir.AluOpType.is_equal)
                            else:
                                oh = oh_shared
                        if "mm" not in parts:
                            continue
                        for j, (w, k, bl, colx, st, sp) in enumerate(chunk):
                            if st:
                                psums[w] = ppool.tile([W, rw], mybir.dt.float32,
                                                      name=f"ps{w}", tag="ps",
                                                      space="PSUM")
                            lhsT = (oh[:, :, j] if OHSRC == "dve"
                                    else oh_g[:, colx - col0, :])
                            nc.tensor.matmul(
                                psums[w][:], lhsT=lhsT,
                                rhs=g_ts[k][:, bl, :rw], start=st, stop=sp,
                            )
                            if sp and "flush" in parts:
                                cop = fpool.tile([W, rw], mybir.dt.float32,
                                                 name=f"cop{w}", tag="cop")
                                nc.scalar.copy(cop[:], psums[w][:])
                                if HALF:
                                    nc.sync.dma_start(
                                        out_d[w * W:(w + 1) * W, :], cop[:])
                                else:
                                    comb = fpool.tile([W, D], mybir.dt.float32,
                                                      name=f"comb{w}", tag="comb")
                                    nc.vector.tensor_tensor(
                                        out=comb[:], in0=cop[:, :D],
                                        in1=cop[:, D:], op=mybir.AluOpType.add,
                                    )
                                    nc.sync.dma_start(
                                        out_d[w * W:(w + 1) * W, :], comb[:])

            if repeat > 1:
                with tc.For_i(0, repeat, 1):
                    body()
            else:
                body()
    nc.finalize()
    return nc


class _SpmdRunner:
    """Compile once, execute the bass kernel across n_cores via PJRT shard_map."""

    def __init__(self, nc, n_cores: int):
        import jax
        import numpy as np
        from jax.experimental.shard_map import shard_map
        from jax.sharding import Mesh, NamedSharding, PartitionSpec
        import concourse.mybir as mybir
        from concourse.bass2jax import (
            _bass_exec_p, install_neuronx_cc_hook, partition_id_tensor,
        )

        install_neuronx_cc_hook()
        self.jax = jax
        self.n_cores = n_cores
        in_names, out_names, out_avals, zero_outs = [], [], [], []
        partition_name = nc.partition_id_tensor.name if nc.partition_id_tensor else None
        for alloc in nc.m.functions[0].allocations:
            if not isinstance(alloc, mybir.MemoryLocationSet):
                continue
            name = alloc.memorylocations[0].name
            if alloc.kind == "ExternalInput":
                if name != partition_name:
                    in_names.append(name)
            elif alloc.kind == "ExternalOutput":
                shape = tuple(alloc.tensor_shape)
                dtype = mybir.dt.np(alloc.dtype)
                out_names.append(name)
                out_avals.append(jax.core.ShapedArray(shape, dtype))
                zero_outs.append(np.zeros(shape, dtype))
        self.n_params = len(in_names)
        self.in_names = list(in_names)
        self.out_names = out_names
        self.out_avals = out_avals
        self.zero_outs = zero_outs
        all_in = in_names + out_names + ([partition_name] if partition_name else [])

        def _body(*args):
            operands = list(args)
            if partition_name is not None:
                operands.append(partition_id_tensor())
            outs = _bass_exec_p.bind(
                *operands,
                out_avals=tuple(out_avals),
                in_names=tuple(all_in),
                out_names=tuple(out_names),
                lowering_input_output_aliases=(),
                sim_require_finite=True,
                sim_require_nnan=True,
                nc=nc,
            )
            return tuple(outs)

        donate = tuple(range(self.n_params, self.n_params + len(out_names)))
        devices = jax.devices()[:n_cores]
        assert len(devices) >= n_cores, f"need {n_cores} cores, got {len(devices)}"
        self.mesh = Mesh(np.asarray(devices), ("core",))
        in_specs = (PartitionSpec("core"),) * (self.n_params + len(out_names))
        out_specs = (PartitionSpec("core"),) * len(out_names)
        self.fn = jax.jit(
            shard_map(_body, mesh=self.mesh, in_specs=in_specs, out_specs=out_specs,
                      check_rep=False),
            donate_argnums=donate,
            keep_unused=True,
        )
        self.sharding = NamedSharding(self.mesh, PartitionSpec("core"))

    def run(self, in_maps):
        np_ = np
        concat_in = [
            np_.concatenate([np_.asarray(in_maps[c][name]) for c in range(self.n_cores)],
                            axis=0)
            for name in self.in_names
        ]
        zeros = [np_.zeros((self.n_cores * z.shape[0], *z.shape[1:]), z.dtype)
                 for z in self.zero_outs]
        out = self.fn(*concat_in, *zeros)
        self.jax.block_until_ready(out)
        return [
            {n: np_.asarray(out[i]).reshape(self.n_cores, *self.out_avals[i].shape)[c]
             for i, n in enumerate(self.out_names)}
            for c in range(self.n_cores)
        ]


_CACHE = {}


def _get_runner(plan):
    if plan.key not in _CACHE:
        nc = _build_nc(plan)
        _CACHE[plan.key] = _SpmdRunner(nc, N_CORES)
    return _CACHE[plan.key]


def _prepare(features: np.ndarray, neigh: np.ndarray, seg: np.ndarray):
    """Returns (plan, in_maps) for the 8 cores."""
    hi = features.astype(ml_dtypes.bfloat16)
    lo = (features - hi.astype(np.float32)).astype(ml_dtypes.bfloat16)
    packed = np.ascontiguousarray(np.concatenate([hi, lo], axis=1))
    plan = _Plan(neigh, seg)
    tabs = {f"tab{k}": packed[k * BUCKET: min((k + 1) * BUCKET, N_NODES)]
            for k in range(NBUK)}
    iotar = np.tile(np.arange(W, dtype=np.float32)[None, :, None],
                    (128, 1, B)).astype(ml_dtypes.bfloat16)
    in_maps = []
    for c in range(N_CORES):
        idx16, extra = plan.core_arrays(c)
        m = dict(tabs)
        m["idx16"] = idx16
        m.update(extra)
        if OHSRC == "dve":
            m["iotar"] = iotar
        in_maps.append(m)
    return plan, in_maps


def kernel(features: np.ndarray, neigh_idx: np.ndarray, seg_ids: np.ndarray,
           ) -> np.ndarray:
    features = np.ascontiguousarray(np.asarray(features, dtype=np.float32))
    neigh = np.asarray(neigh_idx).astype(np.int64)
    seg = np.asarray(seg_ids).astype(np.int64)
    assert features.shape == (N_NODES, D)
    assert neigh.shape == (N_EDGES,) and seg.shape == (N_EDGES,)

    plan, in_maps = _prepare(features, neigh, seg)
    runner = _get_runner(plan)
    results = runner.run(in_maps)
    out = np.empty((N_NODES, D), np.float32)
    for c in range(N_CORES):
        out[c * SEGS_PER_CORE: (c + 1) * SEGS_PER_CORE] = \
            results[c]["out"][:SEGS_PER_CORE]
    return out
